# revision 38
# baseline (speedup 1.0000x reference)
"""Multi-head causal attention kernel for Trainium2 (8 NeuronCores, batch-parallel).

Problem: B=8, Tx=Tz=1024, Dx=Dz=1024, Datt=Dmid=64, H=16, Dout=1024, causal mask.
Sharding: batch dim across the 8 cores (one batch element per core) - weights
replicated, no collectives needed.

Per-core dataflow (all matmul accumulation in fp32 PSUM):
  Fully software-pipelined around the ScalarE exp, which is the rate limiter
  of the attention phase (~82us of ACTIVATE for the causal exp vs ~132us of
  total PE work). The PE instruction queue is in-order, so every spot where
  an attention matmul would wait on exp is padded with dependency-free
  "filler" matmuls: the V-projection chunks and the next head-pair's Q/K
  projections are drip-fed between the S and A@V matmuls.

  schedule: warmup (junk matmuls; HAM clock-gate + DMA-wait cover)
            -> QK(P0) projection (fp8 DoubleRow, 2 contraction tiles per MM)
            -> for P in 0..7: attention(P) with AV lagging S by one j-step,
               interleaved with [V-chunks (P0..P2) then QK(P+1)] fillers
            -> output projection
  V layout: per-(vc, zb) tiles [z=128, 8 heads x 65] with a ones column per
  head (bias via K=1 matmul; sumexp lands in row 64 of the AV psum).
  S^T per (c,j): both heads concurrently via PE row-group tiling; exp does
  both heads in one ACT op; causal-trimmed at 128 granularity, diag masked.
  norm: 1/sumexp via SBUF round-trip (bitwise recip needs fp32-bit layout,
  PSUM holds e10m23) -> partition_broadcast (GPSIMD) -> mul (DVE).
"""
import sys
import types

sys.path.insert(0, "/opt/trn_rl_repo")

# bass_utils imports antenv.axon_hooks when tracing is requested (e.g. via a
# BASS_TRACE env var); that module doesn't exist in this image. Provide a
# no-op stub so tracing degrades gracefully instead of crashing. A test
# harness can pre-register a real hook module before importing this file.
if "antenv.axon_hooks" not in sys.modules:
    _m = types.ModuleType("antenv.axon_hooks")
    _m.get_axon_ntff_profile_hook = lambda: None
    sys.modules["antenv.axon_hooks"] = _m

import ml_dtypes
import numpy as np

import concourse.bacc as bacc
import concourse.mybir as mybir
import concourse.tile as tile
from concourse.bass_utils import run_bass_kernel_spmd

F32 = mybir.dt.float32
FP16 = mybir.dt.float16
FP8 = mybir.dt.float8e4
E4M3 = ml_dtypes.float8_e4m3

B, T, D, E, H = 8, 1024, 1024, 64, 16
NK = D // 128          # 8 contraction tiles
NKP = NK // 2          # 4 fp8 DoubleRow contraction pairs
NP = H // 2            # 8 head pairs
NJ = T // 128          # 8 z tiles
NC = T // 512          # 2 x chunks
SCALE = 0.125          # 1/sqrt(64)


def build_program():
    nc = bacc.Bacc("TRN2", target_bir_lowering=False, debug=False)

    zT_d = nc.dram_tensor("zT", [D, T], FP16, kind="ExternalInput")
    x8_d = nc.dram_tensor("x8", [128, NK * T], FP8, kind="ExternalInput")
    z8_d = nc.dram_tensor("z8", [128, NK * T], FP8, kind="ExternalInput")
    wq_d = nc.dram_tensor("wq", [D, H * E], FP8, kind="ExternalInput")
    wk_d = nc.dram_tensor("wk", [D, H * E], FP8, kind="ExternalInput")
    wv_d = nc.dram_tensor("wv", [D, H * E], FP16, kind="ExternalInput")
    wp_d = nc.dram_tensor("wp", [H * E, D], FP16, kind="ExternalInput")
    bqk_d = nc.dram_tensor("bqk", [128, 16], F32, kind="ExternalInput")
    bvb_d = nc.dram_tensor("bvb", [128, H * E], FP16, kind="ExternalInput")
    bpb_d = nc.dram_tensor("bpb", [128, H * E], F32, kind="ExternalInput")
    maskt_d = nc.dram_tensor("maskt", [128, 256], FP16, kind="ExternalInput")
    out_d = nc.dram_tensor("out", [T, D], FP16, kind="ExternalOutput")

    Exp = mybir.ActivationFunctionType.Exp
    DR = mybir.MatmulPerfMode.DoubleRow

    with tile.TileContext(nc) as tc:
        with (
            tc.tile_pool(name="big", bufs=1) as big,
            tc.tile_pool(name="wf", bufs=2) as wf,
            tc.tile_pool(name="wb", bufs=4) as wb,
            tc.tile_pool(name="qk", bufs=4) as qk,
            tc.tile_pool(name="apool", bufs=12) as apool,
            tc.tile_pool(name="norm", bufs=3) as norm,
            tc.tile_pool(name="opool", bufs=3) as opool,
            tc.tile_pool(name="cst", bufs=1) as cst,
            tc.tile_pool(name="mps", bufs=2, space="PSUM") as mps,
            tc.tile_pool(name="sps", bufs=2, space="PSUM") as sps,
            tc.tile_pool(name="yps", bufs=2, space="PSUM") as yps,
        ):
            # ---- HAM warmup: junk matmuls with no DMA deps keep the PE busy
            # through its 3.4us activity window so real work runs at 2.4 GHz,
            # and cover the DMA wait for the first Q/K inputs ----
            warm_t = cst.tile([128, 512], FP16)
            nc.gpsimd.memset(warm_t[:], 0.0)
            wps = mps.tile([128, 512], F32, tag="mps", name="warmps")
            for _ in range(16):
                nc.tensor.matmul(wps[:], warm_t[:, 0:128], warm_t[:],
                                 start=True, stop=True)

            # ---- resident tiles ----
            bqk_t = cst.tile([128, 16], F32)
            bvb_t = cst.tile([128, H * E], FP16)
            bpb_t = cst.tile([128, H * E], F32)
            maskt_t = cst.tile([128, 256], FP16)
            onesf_t = cst.tile([128, 8], FP16)
            nc.gpsimd.memset(onesf_t[:], 1.0)

            zT_t = [big.tile([128, T], FP16, tag="zTk", bufs=NK, name=f"zT{k}")
                    for k in range(NK)]
            x8_h = [big.tile([128, NK // 2, T], FP8, tag=f"x8{h}",
                             name=f"x8{h}") for h in range(2)]
            z8_h = [big.tile([128, NK // 2, T], FP8, tag=f"z8{h}",
                             name=f"z8{h}") for h in range(2)]
            # V and yT split per chunk so consumers only depend on the pieces
            # they read (dep tracking is tile-granular)
            V_t = [[big.tile([128, 8 * 65], FP16, tag=f"V{vc}_{zb}",
                             name=f"V{vc}_{zb}") for zb in range(NJ)]
                   for vc in range(2)]
            yT_t = [[big.tile([128, 512], FP16, tag=f"yT{P}_{c}",
                              name=f"yT{P}_{c}") for c in range(NC)]
                    for P in range(NP)]
            oh_t = [[big.tile([128, 512], F32, tag=f"oh{dc}_{m}",
                              name=f"oh{dc}_{m}") for m in range(NJ)]
                    for dc in range(2)]
            wv_r = wv_d.ap().rearrange("(k p) he -> p k he", p=128)
            wp_r = wp_d.ap().rearrange("(k p) dout -> p k dout", p=128)
            wq_r = wq_d.ap().rearrange("(k p) he -> p k he", p=128)
            wk_r = wk_d.ap().rearrange("(k p) he -> p k he", p=128)
            zT_r = zT_d.ap().rearrange("(k p) t -> p k t", p=128)

            def fetch_qk_weights(Pn):
                wqP = wb.tile([128, NK, 128], FP8, tag="wb", name=f"wq{Pn}")
                nc.sync.dma_start(wqP[:], wq_r[:, :, Pn * 128:(Pn + 1) * 128])
                wkP = wb.tile([128, NK, 128], FP8, tag="wb", name=f"wk{Pn}")
                nc.sync.dma_start(wkP[:], wk_r[:, :, Pn * 128:(Pn + 1) * 128])
                return wqP, wkP

            # ---- DMA priority order: QK-path inputs first (they gate the
            # attention pipeline), then zT/wv for the V fillers, then consts.
            # x8/z8 arrive host-pre-tiled as [128, NK*T] so each partition
            # line is one contiguous burst.
            half = NK // 2 * T
            nc.sync.dma_start(x8_h[0][:], x8_d.ap()[:, 0:half])
            nc.sync.dma_start(z8_h[0][:], z8_d.ap()[:, 0:half])
            wq0, wk0 = fetch_qk_weights(0)
            nc.sync.dma_start(bqk_t[:], bqk_d.ap())
            nc.sync.dma_start(x8_h[1][:], x8_d.ap()[:, half:2 * half])
            nc.sync.dma_start(z8_h[1][:], z8_d.ap()[:, half:2 * half])
            wvh0 = [wf.tile([128, 512], FP16, tag="wv0", bufs=NK, name=f"wvh0_{k}")
                    for k in range(NK)]
            for k in range(NK):
                nc.sync.dma_start(zT_t[k][:], zT_r[:, k, :])
                nc.sync.dma_start(wvh0[k][:], wv_r[:, k, 0:512])
            nc.sync.dma_start(maskt_t[:], maskt_d.ap())
            nc.sync.dma_start(bvb_t[:], bvb_d.ap())
            nc.sync.dma_start(bpb_t[:], bpb_d.ap())

            # ---- filler thunk factories ----
            def v_chunk_thunks(vc, zb, wvh):
                """V[zb, he-half vc] = zT.T @ Wv-half + bv, plus ones column."""
                state = {}
                th = []
                def alloc(state=state, nm=f"vps{vc}_{zb}"):
                    state["ps"] = mps.tile([128, 512], F32, tag="mps", name=nm)
                def mm(k, vc=vc, zb=zb, wvh=wvh, state=state):
                    rhs = wvh0[k][:] if vc == 0 else wvh[:, k, :]
                    nc.tensor.matmul(
                        state["ps"][:], zT_t[k][:, zb * 128:(zb + 1) * 128],
                        rhs, start=(k == 0), stop=(k == NK - 1),
                        skip_group_check=True,
                    )
                def evict(vc=vc, zb=zb, state=state):
                    dst = V_t[vc][zb][:].rearrange(
                        "p (h c) -> p h c", c=65)[:, :, 0:64]
                    nc.vector.tensor_add(
                        dst, state["ps"][:].rearrange("p (h c) -> p h c", c=64),
                        bvb_t[:, vc * 512:(vc + 1) * 512].rearrange(
                            "p (h c) -> p h c", c=64))
                    ones_dst = V_t[vc][zb][:].rearrange(
                        "p (h c) -> p h c", c=65)[:, :, 64:65]
                    nc.vector.tensor_copy(
                        ones_dst, onesf_t[:].rearrange("p (h c) -> p h c", c=1))
                th.append(alloc)
                for k in range(NK):
                    th.append(lambda k=k, mm=mm: mm(k))
                th.append(evict)
                return th

            def qk_proj_thunks(Pn, wqP, wkP):
                """Per-instruction emission thunks for pair Pn's Q/K proj."""
                QT = qk.tile([128, T], FP16, tag="qk", name=f"QT{Pn}")
                KT = qk.tile([128, T], FP16, tag="qk", name=f"KT{Pn}")
                thunks = []
                for wi, (wt, act, dst, bcol) in enumerate((
                    (wqP, x8_h, QT, Pn), (wkP, z8_h, KT, 8 + Pn),
                )):
                    for c in range(NC):
                        state = {}
                        def alloc(state=state, nm=f"qkps{Pn}_{wi}_{c}"):
                            state["ps"] = mps.tile([128, 512], F32, tag="mps",
                                                   name=nm)
                        def mm(kp, wt=wt, act=act, c=c, state=state):
                            a = act[kp // 2]
                            ko = (kp % 2) * 2
                            nc.tensor.matmul(
                                state["ps"][:], wt[:, 2 * kp:2 * kp + 2, :],
                                a[:, ko:ko + 2, c * 512:(c + 1) * 512],
                                start=(kp == 0), stop=(kp == NKP - 1),
                                perf_mode=DR, skip_group_check=True,
                            )
                        def evict(dst=dst, c=c, bcol=bcol, state=state):
                            nc.vector.tensor_scalar_add(
                                dst[:, c * 512:(c + 1) * 512], state["ps"][:],
                                bqk_t[:, bcol:bcol + 1])
                        thunks.append(alloc)
                        for kp in range(NKP):
                            thunks.append(lambda kp=kp, mm=mm: mm(kp))
                        thunks.append(evict)
                return QT, KT, thunks

            oh_depth = {}

            def out_half_thunks(dc, m, n_ht):
                """Partial out-proj chunk (dc, m): heads 0..n_ht-1, evicted
                to SBUF with the bias folded in; the tail adds the rest."""
                oh_depth[(dc, m)] = n_ht
                state = {}
                th = []
                def alloc(state=state, nm=f"ohps{dc}_{m}"):
                    state["ps"] = mps.tile([128, 512], F32, tag="mps", name=nm)
                def mm(ht, dc=dc, m=m, n_ht=n_ht, state=state):
                    nc.tensor.matmul(
                        state["ps"][:],
                        yT_t[ht][m // 4][:, (m % 4) * 128:(m % 4 + 1) * 128],
                        wph[dc][:, ht, :],
                        start=(ht == 0), stop=(ht == n_ht - 1),
                        skip_group_check=True,
                    )
                def evict(dc=dc, m=m, state=state):
                    nc.vector.tensor_add(
                        oh_t[dc][m][:], state["ps"][:],
                        bpb_t[:, dc * 512:(dc + 1) * 512])
                th.append(alloc)
                for ht in range(n_ht):
                    th.append(lambda ht=ht, mm=mm: mm(ht))
                th.append(evict)
                return th

            def out_tail_thunks(dc, m):
                """Finish out chunk (dc, m): heads 4..7 + prefolded half,
                evict and DMA. Emittable once yT[7] for m's half is done."""
                state = {}
                th = []
                def alloc(state=state, nm=f"otps{dc}_{m}"):
                    state["ps"] = mps.tile([128, 512], F32, tag="mps", name=nm)
                def mm(ht, dc=dc, m=m, state=state):
                    nc.tensor.matmul(
                        state["ps"][:],
                        yT_t[ht][m // 4][:, (m % 4) * 128:(m % 4 + 1) * 128],
                        wph[dc][:, ht, :],
                        start=(ht == NP // 2), stop=(ht == NP - 1),
                        skip_group_check=True,
                    )
                def fin(dc=dc, m=m, state=state):
                    o_t = opool.tile([128, 512], FP16, tag="ot")
                    nc.vector.tensor_add(o_t[:], state["ps"][:], oh_t[dc][m][:])
                    nc.sync.dma_start(
                        out_d.ap()[m * 128:(m + 1) * 128,
                                   dc * 512:(dc + 1) * 512], o_t[:])
                th.append(alloc)
                for ht in range(NP // 2, NP):
                    th.append(lambda ht=ht, mm=mm: mm(ht))
                th.append(fin)
                return th

            OH_SCHED = {4: [(0, 0), (0, 1), (0, 2), (0, 3), (0, 4)],
                        5: [(0, 5), (0, 6), (0, 7), (1, 0), (1, 1)],
                        6: [(1, 2), (1, 3), (1, 4), (1, 5)],
                        7: [(1, 6), (1, 7)]}

            # pair 0's projection runs un-pipelined (first PE work after warmup)
            QT, KT, th0 = qk_proj_thunks(0, wq0, wk0)
            for t in th0:
                t()

            # ---- head-pair loop, software-pipelined ----
            # v_done[vc] counts fully-emitted V chunks of each half; AV for
            # (P, j) requires chunk (P // 4, j).
            v_done = [0, 0]
            for P in range(NP):
                fillers = []
                # stage V-chunk fillers: vc0 during P0, vc1 during P1+P2
                if P == 0:
                    for zb in range(NJ):
                        fillers += v_chunk_thunks(0, zb, None)
                elif P in (1, 2):
                    if P == 1:
                        wvh1 = wf.tile([128, NK, 512], FP16, tag="wf",
                                       name="wvh1")
                        nc.sync.dma_start(wvh1[:], wv_r[:, :, 512:1024])
                    for zb in range(NJ // 2 * (P - 1), NJ // 2 * P):
                        fillers += v_chunk_thunks(1, zb, wvh1)
                elif P == 3:
                    for dc in range(2):
                        w = wf.tile([128, NK, 512], FP16, tag="wf",
                                    name=f"wph{dc}")
                        nc.sync.dma_start(w[:], wp_r[:, :, dc * 512:(dc + 1) * 512])
                        if dc == 0:
                            wph = [w]
                        else:
                            wph.append(w)
                # V-chunk bookkeeping: mark which fillers complete chunks
                v_marks = {}
                n_v = len(fillers)
                if P == 0:
                    for zb in range(NJ):
                        v_marks[(zb + 1) * 10] = (0, zb + 1)
                elif P in (1, 2):
                    base = NJ // 2 * (P - 1)
                    for i in range(NJ // 2):
                        v_marks[(i + 1) * 10] = (1, base + i + 1)

                if P + 1 < NP:
                    wqN, wkN = fetch_qk_weights(P + 1)
                    QTn, KTn, qk_fill = qk_proj_thunks(P + 1, wqN, wkN)
                    fillers += qk_fill
                for dc, m in OH_SCHED.get(P, []):
                    fillers += out_half_thunks(dc, m, NP // 2)
                fill_i = [0]

                def pop_fill(n):
                    for _ in range(n):
                        if fill_i[0] < len(fillers):
                            fillers[fill_i[0]]()
                            fill_i[0] += 1
                            if fill_i[0] in v_marks:
                                vcm, cnt = v_marks[fill_i[0]]
                                v_done[vcm] = cnt

                def pop_until_v(vc, zb):
                    # drain fillers until V chunk (vc, zb) is fully emitted
                    while v_done[vc] <= zb and fill_i[0] < n_v:
                        pop_fill(1)

                seq = [(c, j) for c in range(NC)
                       for j in range(NJ) if 128 * j <= 512 * c + 511]
                quota = -(-len(fillers) // len(seq))  # ceil: spread evenly
                if P == NP - 1:
                    # P7's filler list grows mid-loop (out-proj tail chunks
                    # appended once yT[7,c0] is normalized) - drain eagerly
                    quota = 8
                last_of_c = {c: max(j for cc, j in seq if cc == c)
                             for c in range(NC)}
                yp_t = {}
                pend = []  # [(c, j, at)] awaiting AV (+ norm when last of c)
                vc_P = P // 4

                def emit_av_and_norm(c, j, at):
                    x0 = 128 * max(j - 4 * c, 0)
                    for h01 in range(2):
                        h = 2 * P + h01
                        nc.tensor.matmul(
                            yp_t[c][h01][:, x0:512],
                            V_t[vc_P][j][:, (h % 8) * 65:((h % 8) + 1) * 65],
                            at[:, h01 * 512 + x0:(h01 + 1) * 512],
                            start=(j == 0), stop=(j == last_of_c[c]),
                            skip_group_check=True,
                        )
                    if j == last_of_c[c]:
                        do_tail_append = (P == NP - 1 and c == 0)
                        # normalization + eviction to packed pair layout
                        # (sumexp must round-trip through SBUF:
                        # reciprocal_approx_fast is a bitwise-seed op and
                        # PSUM's e10m23 bits are not fp32)
                        for h01 in range(2):
                            hoff = 64 * h01
                            se_t = norm.tile([1, 512], F32, tag="se")
                            nc.scalar.copy(se_t[:], yp_t[c][h01][64:65, :])
                            r_t = norm.tile([1, 512], F32, tag="rt")
                            nc.vector.reciprocal_approx_fast(r_t[:], se_t[:])
                            bc_t = norm.tile([64, 512], F32, tag="bc")
                            nc.gpsimd.partition_broadcast(bc_t[:], r_t[:])
                            nc.vector.tensor_mul(
                                yT_t[P][c][hoff:hoff + 64, :],
                                yp_t[c][h01][0:64, :], bc_t[:])
                        if do_tail_append:
                            for dcx in range(2):
                                for mx in range(4):
                                    fillers.extend(out_tail_thunks(dcx, mx))

                for (c, j) in seq:
                    if c not in yp_t:
                        yp_t[c] = [yps.tile([65, 512], F32, tag="yps",
                                            name=f"yp{P}_{c}_{h01}")
                                   for h01 in range(2)]
                    kband = j - 4 * c
                    x0 = 128 * max(kband, 0)
                    sp = sps.tile([128, 1024], F32, tag="sps")
                    at = apool.tile([128, 1024], FP16, tag="at")
                    for h01 in range(2):
                        hoff = 64 * h01
                        nc.tensor.matmul(
                            sp[:, h01 * 512 + x0:(h01 + 1) * 512],
                            KT[hoff:hoff + 64, j * 128:(j + 1) * 128],
                            QT[hoff:hoff + 64, c * 512 + x0:(c + 1) * 512],
                            start=True, stop=True, skip_group_check=True,
                        )
                    # one exp over both heads' regions (strided 2-bank AP)
                    sp_v = sp[:].rearrange("p (h x) -> p h x", x=512)[:, :, x0:512]
                    at_v = at[:].rearrange("p (h x) -> p h x", x=512)[:, :, x0:512]
                    nc.scalar.activation(at_v, sp_v, Exp, bias=0.0, scale=SCALE)
                    if kband >= 0:
                        at_m = at[:].rearrange(
                            "p (h x) -> p h x", x=512)[:, :, x0:x0 + 128]
                        mk_m = maskt_t[:].rearrange("p (h x) -> p h x", x=128)
                        nc.vector.tensor_mul(at_m, at_m, mk_m)
                    pop_fill(quota)
                    if len(pend) >= 2:
                        pc, pj, pat = pend.pop(0)
                        pop_until_v(vc_P, pj)
                        if pj == last_of_c[pc]:
                            # extra PE cover for the norm-chain latency the
                            # next chunk's first AV will wait on
                            pop_fill(quota)
                        emit_av_and_norm(pc, pj, pat)
                    pend.append((c, j, at))
                for pc, pj, pat in pend:
                    pop_until_v(vc_P, pj)
                    emit_av_and_norm(pc, pj, pat)
                pop_fill(len(fillers))
                if P + 1 < NP:
                    QT, KT = QTn, KTn

            # ---- output projection tail: remaining m-chunks (the first
            # four per dc were finished inside P7's pipeline) ----
            for dc in range(2):
                for m in range(NJ // 2, NJ):
                    for t in out_tail_thunks(dc, m):
                        t()

    nc.compile()
    return nc


_CACHED_NC = None


def _get_program():
    global _CACHED_NC
    if _CACHED_NC is None:
        _CACHED_NC = build_program()
    return _CACHED_NC


def _prep_shared(Wq, bq, Wk, bk, Wv, bv, Wp, bp, mask):
    assert np.array_equal(
        np.asarray(mask), np.tril(np.ones((T, T), dtype=bool))
    ), "kernel specialized for causal (tril) mask"
    wq = np.ascontiguousarray(
        np.asarray(Wq, np.float32).transpose(1, 0, 2).reshape(D, H * E).astype(E4M3))
    wk = np.ascontiguousarray(
        np.asarray(Wk, np.float32).transpose(1, 0, 2).reshape(D, H * E).astype(E4M3))
    wv = np.ascontiguousarray(
        np.asarray(Wv, np.float32).transpose(1, 0, 2).reshape(D, H * E).astype(np.float16))
    wp = np.ascontiguousarray(np.asarray(Wp, np.float32).astype(np.float16))
    bq_c = np.asarray(bq, np.float32).reshape(-1)
    bk_c = np.asarray(bk, np.float32).reshape(-1)
    bqk = np.concatenate(
        [bq_c.reshape(8, 128).T, bk_c.reshape(8, 128).T], axis=1
    ).astype(np.float32)
    tri = np.triu(np.ones((128, 128), np.float16))  # allow z <= x
    maskt = np.concatenate([tri, tri], axis=1)      # [128, 256] for both heads
    bvb = np.ascontiguousarray(np.broadcast_to(
        np.asarray(bv, np.float32).reshape(1, -1), (128, H * E)).astype(np.float16))
    bpb = np.ascontiguousarray(np.broadcast_to(
        np.asarray(bp, np.float32).reshape(1, -1), (128, H * E)).astype(np.float32))
    return {
        "wq": wq, "wk": wk, "wv": wv, "wp": wp,
        "bqk": np.ascontiguousarray(bqk),
        "bvb": bvb, "bpb": bpb,
        "maskt": np.ascontiguousarray(maskt),
    }


def kernel(x, z, Wq, bq, Wk, bk, Wv, bv, Wp, bp, mask, _trace=False, _trace_kwargs=None):
    x = np.asarray(x, np.float32)
    z = np.asarray(z, np.float32)
    shared = _prep_shared(Wq, bq, Wk, bk, Wv, bv, Wp, bp, mask)
    in_maps = []
    def tile8(a):  # [D,T] -> [128, NK*T] matching the SBUF (p, k, t) layout
        return np.ascontiguousarray(
            a.reshape(NK, 128, T).transpose(1, 0, 2).reshape(128, NK * T)
        ).astype(E4M3)

    for b in range(B):
        m = dict(shared)
        zt = np.ascontiguousarray(z[b].T)
        m["zT"] = zt.astype(np.float16)
        m["z8"] = tile8(zt)
        m["x8"] = tile8(np.ascontiguousarray(x[b].T))
        in_maps.append(m)
    nc = _get_program()
    res = run_bass_kernel_spmd(
        nc, in_maps, core_ids=list(range(B)),
        trace=_trace, **(_trace_kwargs or {}),
    )
    out = np.stack([r["out"] for r in res.results]).astype(np.float32)
    if _trace:
        kernel.last_results = res
    return out


# revision 39
# speedup vs baseline: 1.1890x; 1.1890x over previous
"""Multi-head causal attention kernel for Trainium2 (8 NeuronCores, batch-parallel).

Problem: B=8, Tx=Tz=1024, Dx=Dz=1024, Datt=Dmid=64, H=16, Dout=1024, causal mask.
Sharding: batch dim across the 8 cores (one batch element per core) - weights
replicated, no collectives needed.

Per-core dataflow (all matmul accumulation in fp32 PSUM):
  Fully software-pipelined around the ScalarE exp, which is the rate limiter
  of the attention phase (~82us of ACTIVATE for the causal exp vs ~132us of
  total PE work). The PE instruction queue is in-order, so every spot where
  an attention matmul would wait on exp is padded with dependency-free
  "filler" matmuls: the V-projection chunks and the next head-pair's Q/K
  projections are drip-fed between the S and A@V matmuls.

  schedule: warmup (junk matmuls; HAM clock-gate + DMA-wait cover)
            -> QK(P0) projection (fp8 DoubleRow, 2 contraction tiles per MM)
            -> for P in 0..7: attention(P) with AV lagging S by one j-step,
               interleaved with [V-chunks (P0..P2) then QK(P+1)] fillers
            -> output projection
  V layout: per-(vc, zb) tiles [z=128, 8 heads x 65] with a ones column per
  head (bias via K=1 matmul; sumexp lands in row 64 of the AV psum).
  S^T per (c,j): both heads concurrently via PE row-group tiling; exp does
  both heads in one ACT op; causal-trimmed at 128 granularity, diag masked.
  norm: 1/sumexp via SBUF round-trip (bitwise recip needs fp32-bit layout,
  PSUM holds e10m23) -> partition_broadcast (GPSIMD) -> mul (DVE).
"""
import sys
import types

sys.path.insert(0, "/opt/trn_rl_repo")

# bass_utils imports antenv.axon_hooks when tracing is requested (e.g. via a
# BASS_TRACE env var); that module doesn't exist in this image. Provide a
# no-op stub so tracing degrades gracefully instead of crashing. A test
# harness can pre-register a real hook module before importing this file.
if "antenv.axon_hooks" not in sys.modules:
    _m = types.ModuleType("antenv.axon_hooks")
    _m.get_axon_ntff_profile_hook = lambda: None
    sys.modules["antenv.axon_hooks"] = _m

import ml_dtypes
import numpy as np

import concourse.bacc as bacc
import concourse.mybir as mybir
import concourse.tile as tile
from concourse.bass_utils import run_bass_kernel_spmd

F32 = mybir.dt.float32
FP16 = mybir.dt.float16
FP8 = mybir.dt.float8e4
E4M3 = ml_dtypes.float8_e4m3

B, T, D, E, H = 8, 1024, 1024, 64, 16
NK = D // 128          # 8 contraction tiles
NKP = NK // 2          # 4 fp8 DoubleRow contraction pairs
NP = H // 2            # 8 head pairs
NJ = T // 128          # 8 z tiles
NC = T // 512          # 2 x chunks
SCALE = 0.125          # 1/sqrt(64)


def build_program():
    nc = bacc.Bacc("TRN2", target_bir_lowering=False, debug=False)

    zT_d = nc.dram_tensor("zT", [D, T], FP16, kind="ExternalInput")
    x8_d = nc.dram_tensor("x8", [128, NK * T], FP8, kind="ExternalInput")
    z8_d = nc.dram_tensor("z8", [128, NK * T], FP8, kind="ExternalInput")
    wq_d = nc.dram_tensor("wq", [D, H * E], FP8, kind="ExternalInput")
    wk_d = nc.dram_tensor("wk", [D, H * E], FP8, kind="ExternalInput")
    wv_d = nc.dram_tensor("wv", [D, H * E], FP16, kind="ExternalInput")
    wp_d = nc.dram_tensor("wp", [H * E, D], FP16, kind="ExternalInput")
    bqk_d = nc.dram_tensor("bqk", [128, 16], F32, kind="ExternalInput")
    bvb_d = nc.dram_tensor("bvb", [128, H * E], FP16, kind="ExternalInput")
    bpb_d = nc.dram_tensor("bpb", [128, H * E], F32, kind="ExternalInput")
    maskt_d = nc.dram_tensor("maskt", [128, 256], FP16, kind="ExternalInput")
    out_d = nc.dram_tensor("out", [T, D], FP16, kind="ExternalOutput")

    Exp = mybir.ActivationFunctionType.Exp
    DR = mybir.MatmulPerfMode.DoubleRow

    with tile.TileContext(nc) as tc:
        with (
            tc.tile_pool(name="big", bufs=1) as big,
            tc.tile_pool(name="wf", bufs=2) as wf,
            tc.tile_pool(name="wb", bufs=4) as wb,
            tc.tile_pool(name="qk", bufs=4) as qk,
            tc.tile_pool(name="apool", bufs=12) as apool,
            tc.tile_pool(name="norm", bufs=3) as norm,
            tc.tile_pool(name="opool", bufs=3) as opool,
            tc.tile_pool(name="cst", bufs=1) as cst,
            tc.tile_pool(name="mps", bufs=2, space="PSUM") as mps,
            tc.tile_pool(name="sps", bufs=2, space="PSUM") as sps,
            tc.tile_pool(name="yps", bufs=2, space="PSUM") as yps,
        ):
            # ---- HAM warmup: junk matmuls with no DMA deps keep the PE busy
            # through its 3.4us activity window so real work runs at 2.4 GHz,
            # and cover the DMA wait for the first Q/K inputs ----
            warm_t = cst.tile([128, 512], FP16)
            nc.gpsimd.memset(warm_t[:], 0.0)
            wps = mps.tile([128, 512], F32, tag="mps", name="warmps")
            for _ in range(16):
                nc.tensor.matmul(wps[:], warm_t[:, 0:128], warm_t[:],
                                 start=True, stop=True)

            # ---- resident tiles ----
            bqk_t = cst.tile([128, 16], F32)
            bvb_t = cst.tile([128, H * E], FP16)
            bpb_t = cst.tile([128, H * E], F32)
            maskt_t = cst.tile([128, 256], FP16)
            onesf_t = cst.tile([128, 8], FP16)
            nc.gpsimd.memset(onesf_t[:], 1.0)

            zT_t = [big.tile([128, T], FP16, tag="zTk", bufs=NK, name=f"zT{k}")
                    for k in range(NK)]
            x8_h = [big.tile([128, NK // 2, T], FP8, tag=f"x8{h}",
                             name=f"x8{h}") for h in range(2)]
            z8_h = [big.tile([128, NK // 2, T], FP8, tag=f"z8{h}",
                             name=f"z8{h}") for h in range(2)]
            # V and yT split per chunk so consumers only depend on the pieces
            # they read (dep tracking is tile-granular)
            V_t = [[big.tile([128, 8 * 65], FP16, tag=f"V{vc}_{zb}",
                             name=f"V{vc}_{zb}") for zb in range(NJ)]
                   for vc in range(2)]
            yT_t = [big.tile([128, T], FP16, tag=f"yT{P}", name=f"yT{P}")
                    for P in range(NP)]
            oh_t = [[big.tile([128, 512], F32, tag=f"oh{dc}_{m}",
                              name=f"oh{dc}_{m}") for m in range(NJ)]
                    for dc in range(2)]
            wv_r = wv_d.ap().rearrange("(k p) he -> p k he", p=128)
            wp_r = wp_d.ap().rearrange("(k p) dout -> p k dout", p=128)
            wq_r = wq_d.ap().rearrange("(k p) he -> p k he", p=128)
            wk_r = wk_d.ap().rearrange("(k p) he -> p k he", p=128)
            zT_r = zT_d.ap().rearrange("(k p) t -> p k t", p=128)

            def fetch_qk_weights(Pn):
                wqP = wb.tile([128, NK, 128], FP8, tag="wb", name=f"wq{Pn}")
                nc.sync.dma_start(wqP[:], wq_r[:, :, Pn * 128:(Pn + 1) * 128])
                wkP = wb.tile([128, NK, 128], FP8, tag="wb", name=f"wk{Pn}")
                nc.sync.dma_start(wkP[:], wk_r[:, :, Pn * 128:(Pn + 1) * 128])
                return wqP, wkP

            # ---- DMA priority order: QK-path inputs first (they gate the
            # attention pipeline), then zT/wv for the V fillers, then consts.
            # x8/z8 arrive host-pre-tiled as [128, NK*T] so each partition
            # line is one contiguous burst.
            half = NK // 2 * T
            nc.sync.dma_start(x8_h[0][:], x8_d.ap()[:, 0:half])
            nc.sync.dma_start(z8_h[0][:], z8_d.ap()[:, 0:half])
            wq0, wk0 = fetch_qk_weights(0)
            nc.sync.dma_start(bqk_t[:], bqk_d.ap())
            nc.sync.dma_start(x8_h[1][:], x8_d.ap()[:, half:2 * half])
            nc.sync.dma_start(z8_h[1][:], z8_d.ap()[:, half:2 * half])
            wvh0 = [wf.tile([128, 512], FP16, tag="wv0", bufs=NK, name=f"wvh0_{k}")
                    for k in range(NK)]
            for k in range(NK):
                nc.sync.dma_start(zT_t[k][:], zT_r[:, k, :])
                nc.sync.dma_start(wvh0[k][:], wv_r[:, k, 0:512])
            nc.sync.dma_start(maskt_t[:], maskt_d.ap())
            nc.sync.dma_start(bvb_t[:], bvb_d.ap())
            nc.sync.dma_start(bpb_t[:], bpb_d.ap())

            # ---- filler thunk factories ----
            def v_chunk_thunks(vc, zb, wvh):
                """V[zb, he-half vc] = zT.T @ Wv-half + bv, plus ones column."""
                state = {}
                th = []
                def alloc(state=state, nm=f"vps{vc}_{zb}"):
                    state["ps"] = mps.tile([128, 512], F32, tag="mps", name=nm)
                def mm(k, vc=vc, zb=zb, wvh=wvh, state=state):
                    rhs = wvh0[k][:] if vc == 0 else wvh[:, k, :]
                    nc.tensor.matmul(
                        state["ps"][:], zT_t[k][:, zb * 128:(zb + 1) * 128],
                        rhs, start=(k == 0), stop=(k == NK - 1),
                        skip_group_check=True,
                    )
                def evict(vc=vc, zb=zb, state=state):
                    dst = V_t[vc][zb][:].rearrange(
                        "p (h c) -> p h c", c=65)[:, :, 0:64]
                    nc.vector.tensor_add(
                        dst, state["ps"][:].rearrange("p (h c) -> p h c", c=64),
                        bvb_t[:, vc * 512:(vc + 1) * 512].rearrange(
                            "p (h c) -> p h c", c=64))
                    ones_dst = V_t[vc][zb][:].rearrange(
                        "p (h c) -> p h c", c=65)[:, :, 64:65]
                    nc.vector.tensor_copy(
                        ones_dst, onesf_t[:].rearrange("p (h c) -> p h c", c=1))
                th.append(alloc)
                for k in range(NK):
                    th.append(lambda k=k, mm=mm: mm(k))
                th.append(evict)
                return th

            def qk_proj_thunks(Pn, wqP, wkP):
                """Per-instruction emission thunks for pair Pn's Q/K proj."""
                QT = qk.tile([128, T], FP16, tag="qk", name=f"QT{Pn}")
                KT = qk.tile([128, T], FP16, tag="qk", name=f"KT{Pn}")
                thunks = []
                for wi, (wt, act, dst, bcol) in enumerate((
                    (wqP, x8_h, QT, Pn), (wkP, z8_h, KT, 8 + Pn),
                )):
                    for c in range(NC):
                        state = {}
                        def alloc(state=state, nm=f"qkps{Pn}_{wi}_{c}"):
                            state["ps"] = mps.tile([128, 512], F32, tag="mps",
                                                   name=nm)
                        def mm(kp, wt=wt, act=act, c=c, state=state):
                            a = act[kp // 2]
                            ko = (kp % 2) * 2
                            nc.tensor.matmul(
                                state["ps"][:], wt[:, 2 * kp:2 * kp + 2, :],
                                a[:, ko:ko + 2, c * 512:(c + 1) * 512],
                                start=(kp == 0), stop=(kp == NKP - 1),
                                perf_mode=DR, skip_group_check=True,
                            )
                        def evict(dst=dst, c=c, bcol=bcol, state=state):
                            nc.vector.tensor_scalar_add(
                                dst[:, c * 512:(c + 1) * 512], state["ps"][:],
                                bqk_t[:, bcol:bcol + 1])
                        thunks.append(alloc)
                        for kp in range(NKP):
                            thunks.append(lambda kp=kp, mm=mm: mm(kp))
                        thunks.append(evict)
                return QT, KT, thunks

            oh_depth = {}

            def out_half_thunks(dc, m, n_ht):
                """Partial out-proj chunk (dc, m): heads 0..n_ht-1, evicted
                to SBUF with the bias folded in; the tail adds the rest."""
                oh_depth[(dc, m)] = n_ht
                state = {}
                th = []
                def alloc(state=state, nm=f"ohps{dc}_{m}"):
                    state["ps"] = mps.tile([128, 512], F32, tag="mps", name=nm)
                def mm(ht, dc=dc, m=m, n_ht=n_ht, state=state):
                    nc.tensor.matmul(
                        state["ps"][:], yT_t[ht][:, m * 128:(m + 1) * 128],
                        wph[dc][:, ht, :],
                        start=(ht == 0), stop=(ht == n_ht - 1),
                        skip_group_check=True,
                    )
                def evict(dc=dc, m=m, state=state):
                    nc.vector.tensor_add(
                        oh_t[dc][m][:], state["ps"][:],
                        bpb_t[:, dc * 512:(dc + 1) * 512])
                th.append(alloc)
                for ht in range(n_ht):
                    th.append(lambda ht=ht, mm=mm: mm(ht))
                th.append(evict)
                return th

            OH_SCHED = {4: [(0, 0), (0, 1), (0, 2), (0, 3), (0, 4)],
                        5: [(0, 5), (0, 6), (0, 7), (1, 0), (1, 1)],
                        6: [(1, 2), (1, 3), (1, 4), (1, 5)],
                        7: [(1, 6), (1, 7)]}

            # pair 0's projection runs un-pipelined (first PE work after warmup)
            QT, KT, th0 = qk_proj_thunks(0, wq0, wk0)
            for t in th0:
                t()

            # ---- head-pair loop, software-pipelined ----
            # v_done[vc] counts fully-emitted V chunks of each half; AV for
            # (P, j) requires chunk (P // 4, j).
            v_done = [0, 0]
            for P in range(NP):
                fillers = []
                # stage V-chunk fillers: vc0 during P0, vc1 during P1+P2
                if P == 0:
                    for zb in range(NJ):
                        fillers += v_chunk_thunks(0, zb, None)
                elif P in (1, 2):
                    if P == 1:
                        wvh1 = wf.tile([128, NK, 512], FP16, tag="wf",
                                       name="wvh1")
                        nc.sync.dma_start(wvh1[:], wv_r[:, :, 512:1024])
                    for zb in range(NJ // 2 * (P - 1), NJ // 2 * P):
                        fillers += v_chunk_thunks(1, zb, wvh1)
                elif P == 3:
                    for dc in range(2):
                        w = wf.tile([128, NK, 512], FP16, tag="wf",
                                    name=f"wph{dc}")
                        nc.sync.dma_start(w[:], wp_r[:, :, dc * 512:(dc + 1) * 512])
                        if dc == 0:
                            wph = [w]
                        else:
                            wph.append(w)
                # V-chunk bookkeeping: mark which fillers complete chunks
                v_marks = {}
                n_v = len(fillers)
                if P == 0:
                    for zb in range(NJ):
                        v_marks[(zb + 1) * 10] = (0, zb + 1)
                elif P in (1, 2):
                    base = NJ // 2 * (P - 1)
                    for i in range(NJ // 2):
                        v_marks[(i + 1) * 10] = (1, base + i + 1)

                if P + 1 < NP:
                    wqN, wkN = fetch_qk_weights(P + 1)
                    QTn, KTn, qk_fill = qk_proj_thunks(P + 1, wqN, wkN)
                    fillers += qk_fill
                for dc, m in OH_SCHED.get(P, []):
                    fillers += out_half_thunks(dc, m, NP // 2)
                fill_i = [0]

                def pop_fill(n):
                    for _ in range(n):
                        if fill_i[0] < len(fillers):
                            fillers[fill_i[0]]()
                            fill_i[0] += 1
                            if fill_i[0] in v_marks:
                                vcm, cnt = v_marks[fill_i[0]]
                                v_done[vcm] = cnt

                def pop_until_v(vc, zb):
                    # drain fillers until V chunk (vc, zb) is fully emitted
                    while v_done[vc] <= zb and fill_i[0] < n_v:
                        pop_fill(1)

                seq = [(c, j) for c in range(NC)
                       for j in range(NJ) if 128 * j <= 512 * c + 511]
                quota = -(-len(fillers) // len(seq))  # ceil: spread evenly
                last_of_c = {c: max(j for cc, j in seq if cc == c)
                             for c in range(NC)}
                yp_t = {}
                pend = []  # [(c, j, at)] awaiting AV (+ norm when last of c)
                vc_P = P // 4

                def emit_av_and_norm(c, j, at):
                    x0 = 128 * max(j - 4 * c, 0)
                    for h01 in range(2):
                        h = 2 * P + h01
                        nc.tensor.matmul(
                            yp_t[c][h01][:, x0:512],
                            V_t[vc_P][j][:, (h % 8) * 65:((h % 8) + 1) * 65],
                            at[:, h01 * 512 + x0:(h01 + 1) * 512],
                            start=(j == 0), stop=(j == last_of_c[c]),
                            skip_group_check=True,
                        )
                    if j == last_of_c[c]:
                        # normalization + eviction to packed pair layout
                        # (sumexp must round-trip through SBUF:
                        # reciprocal_approx_fast is a bitwise-seed op and
                        # PSUM's e10m23 bits are not fp32)
                        for h01 in range(2):
                            hoff = 64 * h01
                            se_t = norm.tile([1, 512], F32, tag="se")
                            nc.scalar.copy(se_t[:], yp_t[c][h01][64:65, :])
                            r_t = norm.tile([1, 512], F32, tag="rt")
                            nc.vector.reciprocal_approx_fast(r_t[:], se_t[:])
                            bc_t = norm.tile([64, 512], F32, tag="bc")
                            nc.gpsimd.partition_broadcast(bc_t[:], r_t[:])
                            nc.vector.tensor_mul(
                                yT_t[P][hoff:hoff + 64, c * 512:(c + 1) * 512],
                                yp_t[c][h01][0:64, :], bc_t[:])

                for (c, j) in seq:
                    if c not in yp_t:
                        yp_t[c] = [yps.tile([65, 512], F32, tag="yps",
                                            name=f"yp{P}_{c}_{h01}")
                                   for h01 in range(2)]
                    kband = j - 4 * c
                    x0 = 128 * max(kband, 0)
                    sp = sps.tile([128, 1024], F32, tag="sps")
                    at = apool.tile([128, 1024], FP16, tag="at")
                    for h01 in range(2):
                        hoff = 64 * h01
                        nc.tensor.matmul(
                            sp[:, h01 * 512 + x0:(h01 + 1) * 512],
                            KT[hoff:hoff + 64, j * 128:(j + 1) * 128],
                            QT[hoff:hoff + 64, c * 512 + x0:(c + 1) * 512],
                            start=True, stop=True, skip_group_check=True,
                        )
                    # one exp over both heads' regions (strided 2-bank AP)
                    sp_v = sp[:].rearrange("p (h x) -> p h x", x=512)[:, :, x0:512]
                    at_v = at[:].rearrange("p (h x) -> p h x", x=512)[:, :, x0:512]
                    nc.scalar.activation(at_v, sp_v, Exp, bias=0.0, scale=SCALE)
                    if kband >= 0:
                        at_m = at[:].rearrange(
                            "p (h x) -> p h x", x=512)[:, :, x0:x0 + 128]
                        mk_m = maskt_t[:].rearrange("p (h x) -> p h x", x=128)
                        nc.vector.tensor_mul(at_m, at_m, mk_m)
                    pop_fill(quota)
                    if len(pend) >= 2:
                        pc, pj, pat = pend.pop(0)
                        pop_until_v(vc_P, pj)
                        if pj == last_of_c[pc]:
                            # extra PE cover for the norm-chain latency the
                            # next chunk's first AV will wait on
                            pop_fill(quota)
                        emit_av_and_norm(pc, pj, pat)
                    pend.append((c, j, at))
                for pc, pj, pat in pend:
                    pop_until_v(vc_P, pj)
                    emit_av_and_norm(pc, pj, pat)
                pop_fill(len(fillers))
                if P + 1 < NP:
                    QT, KT = QTn, KTn

            # ---- output projection tail: ht 4..7 plus the prefolded half ----
            for dc in range(2):
                for m in range(NJ):
                    ps = mps.tile([128, 512], F32, tag="mps")
                    for ht in range(NP // 2, NP):
                        nc.tensor.matmul(
                            ps[:], yT_t[ht][:, m * 128:(m + 1) * 128], wph[dc][:, ht, :],
                            start=(ht == NP // 2), stop=(ht == NP - 1),
                        )
                    o_t = opool.tile([128, 512], FP16, tag="ot")
                    nc.vector.tensor_add(o_t[:], ps[:], oh_t[dc][m][:])
                    nc.sync.dma_start(
                        out_d.ap()[m * 128:(m + 1) * 128, dc * 512:(dc + 1) * 512],
                        o_t[:])

    nc.compile()
    return nc


_CACHED_NC = None


def _get_program():
    global _CACHED_NC
    if _CACHED_NC is None:
        _CACHED_NC = build_program()
    return _CACHED_NC


def _prep_shared(Wq, bq, Wk, bk, Wv, bv, Wp, bp, mask):
    assert np.array_equal(
        np.asarray(mask), np.tril(np.ones((T, T), dtype=bool))
    ), "kernel specialized for causal (tril) mask"
    wq = np.ascontiguousarray(
        np.asarray(Wq, np.float32).transpose(1, 0, 2).reshape(D, H * E).astype(E4M3))
    wk = np.ascontiguousarray(
        np.asarray(Wk, np.float32).transpose(1, 0, 2).reshape(D, H * E).astype(E4M3))
    wv = np.ascontiguousarray(
        np.asarray(Wv, np.float32).transpose(1, 0, 2).reshape(D, H * E).astype(np.float16))
    wp = np.ascontiguousarray(np.asarray(Wp, np.float32).astype(np.float16))
    bq_c = np.asarray(bq, np.float32).reshape(-1)
    bk_c = np.asarray(bk, np.float32).reshape(-1)
    bqk = np.concatenate(
        [bq_c.reshape(8, 128).T, bk_c.reshape(8, 128).T], axis=1
    ).astype(np.float32)
    tri = np.triu(np.ones((128, 128), np.float16))  # allow z <= x
    maskt = np.concatenate([tri, tri], axis=1)      # [128, 256] for both heads
    bvb = np.ascontiguousarray(np.broadcast_to(
        np.asarray(bv, np.float32).reshape(1, -1), (128, H * E)).astype(np.float16))
    bpb = np.ascontiguousarray(np.broadcast_to(
        np.asarray(bp, np.float32).reshape(1, -1), (128, H * E)).astype(np.float32))
    return {
        "wq": wq, "wk": wk, "wv": wv, "wp": wp,
        "bqk": np.ascontiguousarray(bqk),
        "bvb": bvb, "bpb": bpb,
        "maskt": np.ascontiguousarray(maskt),
    }


def kernel(x, z, Wq, bq, Wk, bk, Wv, bv, Wp, bp, mask, _trace=False, _trace_kwargs=None):
    x = np.asarray(x, np.float32)
    z = np.asarray(z, np.float32)
    shared = _prep_shared(Wq, bq, Wk, bk, Wv, bv, Wp, bp, mask)
    in_maps = []
    def tile8(a):  # [D,T] -> [128, NK*T] matching the SBUF (p, k, t) layout
        return np.ascontiguousarray(
            a.reshape(NK, 128, T).transpose(1, 0, 2).reshape(128, NK * T)
        ).astype(E4M3)

    for b in range(B):
        m = dict(shared)
        zt = np.ascontiguousarray(z[b].T)
        m["zT"] = zt.astype(np.float16)
        m["z8"] = tile8(zt)
        m["x8"] = tile8(np.ascontiguousarray(x[b].T))
        in_maps.append(m)
    nc = _get_program()
    res = run_bass_kernel_spmd(
        nc, in_maps, core_ids=list(range(B)),
        trace=_trace, **(_trace_kwargs or {}),
    )
    out = np.stack([r["out"] for r in res.results]).astype(np.float32)
    if _trace:
        kernel.last_results = res
    return out


# revision 40
# speedup vs baseline: 1.1951x; 1.0051x over previous
"""Multi-head causal attention kernel for Trainium2 (8 NeuronCores, batch-parallel).

Problem: B=8, Tx=Tz=1024, Dx=Dz=1024, Datt=Dmid=64, H=16, Dout=1024, causal mask.
Sharding: batch dim across the 8 cores (one batch element per core) - weights
replicated, no collectives needed.

Per-core dataflow (all matmul accumulation in fp32 PSUM):
  Fully software-pipelined around the ScalarE exp, which is the rate limiter
  of the attention phase (~82us of ACTIVATE for the causal exp vs ~132us of
  total PE work). The PE instruction queue is in-order, so every spot where
  an attention matmul would wait on exp is padded with dependency-free
  "filler" matmuls: the V-projection chunks and the next head-pair's Q/K
  projections are drip-fed between the S and A@V matmuls.

  schedule: warmup (junk matmuls; HAM clock-gate + DMA-wait cover)
            -> QK(P0) projection (fp8 DoubleRow, 2 contraction tiles per MM)
            -> for P in 0..7: attention(P) with AV lagging S by one j-step,
               interleaved with [V-chunks (P0..P2) then QK(P+1)] fillers
            -> output projection
  V layout: per-(vc, zb) tiles [z=128, 8 heads x 65] with a ones column per
  head (bias via K=1 matmul; sumexp lands in row 64 of the AV psum).
  S^T per (c,j): both heads concurrently via PE row-group tiling; exp does
  both heads in one ACT op; causal-trimmed at 128 granularity, diag masked.
  norm: 1/sumexp via SBUF round-trip (bitwise recip needs fp32-bit layout,
  PSUM holds e10m23) -> partition_broadcast (GPSIMD) -> mul (DVE).
"""
import sys
import types

sys.path.insert(0, "/opt/trn_rl_repo")

# bass_utils imports antenv.axon_hooks when tracing is requested (e.g. via a
# BASS_TRACE env var); that module doesn't exist in this image. Provide a
# no-op stub so tracing degrades gracefully instead of crashing. A test
# harness can pre-register a real hook module before importing this file.
if "antenv.axon_hooks" not in sys.modules:
    _m = types.ModuleType("antenv.axon_hooks")
    _m.get_axon_ntff_profile_hook = lambda: None
    sys.modules["antenv.axon_hooks"] = _m

import ml_dtypes
import numpy as np

import concourse.bacc as bacc
import concourse.mybir as mybir
import concourse.tile as tile
from concourse.bass_utils import run_bass_kernel_spmd

F32 = mybir.dt.float32
FP16 = mybir.dt.float16
FP8 = mybir.dt.float8e4
E4M3 = ml_dtypes.float8_e4m3

B, T, D, E, H = 8, 1024, 1024, 64, 16
NK = D // 128          # 8 contraction tiles
NKP = NK // 2          # 4 fp8 DoubleRow contraction pairs
NP = H // 2            # 8 head pairs
NJ = T // 128          # 8 z tiles
NC = T // 512          # 2 x chunks
SCALE = 0.125          # 1/sqrt(64)


def build_program():
    nc = bacc.Bacc("TRN2", target_bir_lowering=False, debug=False)

    zT_d = nc.dram_tensor("zT", [D, T], FP16, kind="ExternalInput")
    x8_d = nc.dram_tensor("x8", [128, NK * T], FP8, kind="ExternalInput")
    z8_d = nc.dram_tensor("z8", [128, NK * T], FP8, kind="ExternalInput")
    wq_d = nc.dram_tensor("wq", [D, H * E], FP8, kind="ExternalInput")
    wk_d = nc.dram_tensor("wk", [D, H * E], FP8, kind="ExternalInput")
    wv_d = nc.dram_tensor("wv", [D, H * E], FP16, kind="ExternalInput")
    wp_d = nc.dram_tensor("wp", [H * E, D], FP16, kind="ExternalInput")
    bqk_d = nc.dram_tensor("bqk", [128, 16], F32, kind="ExternalInput")
    bvb_d = nc.dram_tensor("bvb", [128, H * E], FP16, kind="ExternalInput")
    bpb_d = nc.dram_tensor("bpb", [128, H * E], F32, kind="ExternalInput")
    maskt_d = nc.dram_tensor("maskt", [128, 256], FP16, kind="ExternalInput")
    out_d = nc.dram_tensor("out", [T, D], FP16, kind="ExternalOutput")

    Exp = mybir.ActivationFunctionType.Exp
    DR = mybir.MatmulPerfMode.DoubleRow

    with tile.TileContext(nc) as tc:
        with (
            tc.tile_pool(name="big", bufs=1) as big,
            tc.tile_pool(name="wf", bufs=2) as wf,
            tc.tile_pool(name="wb", bufs=4) as wb,
            tc.tile_pool(name="qk", bufs=4) as qk,
            tc.tile_pool(name="apool", bufs=12) as apool,
            tc.tile_pool(name="norm", bufs=3) as norm,
            tc.tile_pool(name="opool", bufs=3) as opool,
            tc.tile_pool(name="cst", bufs=1) as cst,
            tc.tile_pool(name="mps", bufs=2, space="PSUM") as mps,
            tc.tile_pool(name="sps", bufs=2, space="PSUM") as sps,
            tc.tile_pool(name="yps", bufs=2, space="PSUM") as yps,
        ):
            # ---- HAM warmup: junk matmuls with no DMA deps keep the PE busy
            # through its 3.4us activity window so real work runs at 2.4 GHz,
            # and cover the DMA wait for the first Q/K inputs ----
            warm_t = cst.tile([128, 512], FP16)
            nc.gpsimd.memset(warm_t[:], 0.0)
            wps = mps.tile([128, 512], F32, tag="mps", name="warmps")
            for _ in range(20):
                nc.tensor.matmul(wps[:], warm_t[:, 0:128], warm_t[:],
                                 start=True, stop=True)

            # ---- resident tiles ----
            bqk_t = cst.tile([128, 16], F32)
            bvb_t = cst.tile([128, H * E], FP16)
            bpb_t = cst.tile([128, H * E], F32)
            maskt_t = cst.tile([128, 256], FP16)
            onesf_t = cst.tile([128, 8], FP16)
            nc.gpsimd.memset(onesf_t[:], 1.0)

            zT_t = [big.tile([128, T], FP16, tag="zTk", bufs=NK, name=f"zT{k}")
                    for k in range(NK)]
            x8_h = [big.tile([128, NK // 2, T], FP8, tag=f"x8{h}",
                             name=f"x8{h}") for h in range(2)]
            z8_h = [big.tile([128, NK // 2, T], FP8, tag=f"z8{h}",
                             name=f"z8{h}") for h in range(2)]
            # V and yT split per chunk so consumers only depend on the pieces
            # they read (dep tracking is tile-granular)
            V_t = [[big.tile([128, 8 * 65], FP16, tag=f"V{vc}_{zb}",
                             name=f"V{vc}_{zb}") for zb in range(NJ)]
                   for vc in range(2)]
            yT_t = [big.tile([128, T], FP16, tag=f"yT{P}", name=f"yT{P}")
                    for P in range(NP)]
            oh_t = [[big.tile([128, 512], F32, tag=f"oh{dc}_{m}",
                              name=f"oh{dc}_{m}") for m in range(NJ)]
                    for dc in range(2)]
            wv_r = wv_d.ap().rearrange("(k p) he -> p k he", p=128)
            wp_r = wp_d.ap().rearrange("(k p) dout -> p k dout", p=128)
            wq_r = wq_d.ap().rearrange("(k p) he -> p k he", p=128)
            wk_r = wk_d.ap().rearrange("(k p) he -> p k he", p=128)
            zT_r = zT_d.ap().rearrange("(k p) t -> p k t", p=128)

            def fetch_qk_weights(Pn):
                wqP = wb.tile([128, NK, 128], FP8, tag="wb", name=f"wq{Pn}")
                nc.sync.dma_start(wqP[:], wq_r[:, :, Pn * 128:(Pn + 1) * 128])
                wkP = wb.tile([128, NK, 128], FP8, tag="wb", name=f"wk{Pn}")
                nc.sync.dma_start(wkP[:], wk_r[:, :, Pn * 128:(Pn + 1) * 128])
                return wqP, wkP

            # ---- DMA priority order: QK-path inputs first (they gate the
            # attention pipeline), then zT/wv for the V fillers, then consts.
            # x8/z8 arrive host-pre-tiled as [128, NK*T] so each partition
            # line is one contiguous burst.
            half = NK // 2 * T
            nc.sync.dma_start(x8_h[0][:], x8_d.ap()[:, 0:half])
            nc.sync.dma_start(z8_h[0][:], z8_d.ap()[:, 0:half])
            wq0, wk0 = fetch_qk_weights(0)
            nc.sync.dma_start(bqk_t[:], bqk_d.ap())
            nc.sync.dma_start(x8_h[1][:], x8_d.ap()[:, half:2 * half])
            nc.sync.dma_start(z8_h[1][:], z8_d.ap()[:, half:2 * half])
            wvh0 = [wf.tile([128, 512], FP16, tag="wv0", bufs=NK, name=f"wvh0_{k}")
                    for k in range(NK)]
            for k in range(NK):
                nc.sync.dma_start(zT_t[k][:], zT_r[:, k, :])
                nc.sync.dma_start(wvh0[k][:], wv_r[:, k, 0:512])
            nc.sync.dma_start(maskt_t[:], maskt_d.ap())
            nc.sync.dma_start(bvb_t[:], bvb_d.ap())
            nc.sync.dma_start(bpb_t[:], bpb_d.ap())

            # ---- filler thunk factories ----
            def v_chunk_thunks(vc, zb, wvh):
                """V[zb, he-half vc] = zT.T @ Wv-half + bv, plus ones column."""
                state = {}
                th = []
                def alloc(state=state, nm=f"vps{vc}_{zb}"):
                    state["ps"] = mps.tile([128, 512], F32, tag="mps", name=nm)
                def mm(k, vc=vc, zb=zb, wvh=wvh, state=state):
                    rhs = wvh0[k][:] if vc == 0 else wvh[:, k, :]
                    nc.tensor.matmul(
                        state["ps"][:], zT_t[k][:, zb * 128:(zb + 1) * 128],
                        rhs, start=(k == 0), stop=(k == NK - 1),
                        skip_group_check=True,
                    )
                def evict(vc=vc, zb=zb, state=state):
                    dst = V_t[vc][zb][:].rearrange(
                        "p (h c) -> p h c", c=65)[:, :, 0:64]
                    nc.vector.tensor_add(
                        dst, state["ps"][:].rearrange("p (h c) -> p h c", c=64),
                        bvb_t[:, vc * 512:(vc + 1) * 512].rearrange(
                            "p (h c) -> p h c", c=64))
                    ones_dst = V_t[vc][zb][:].rearrange(
                        "p (h c) -> p h c", c=65)[:, :, 64:65]
                    nc.vector.tensor_copy(
                        ones_dst, onesf_t[:].rearrange("p (h c) -> p h c", c=1))
                th.append(alloc)
                for k in range(NK):
                    th.append(lambda k=k, mm=mm: mm(k))
                th.append(evict)
                return th

            def qk_proj_thunks(Pn, wqP, wkP):
                """Per-instruction emission thunks for pair Pn's Q/K proj."""
                QT = qk.tile([128, T], FP16, tag="qk", name=f"QT{Pn}")
                KT = qk.tile([128, T], FP16, tag="qk", name=f"KT{Pn}")
                thunks = []
                for wi, (wt, act, dst, bcol) in enumerate((
                    (wqP, x8_h, QT, Pn), (wkP, z8_h, KT, 8 + Pn),
                )):
                    for c in range(NC):
                        state = {}
                        def alloc(state=state, nm=f"qkps{Pn}_{wi}_{c}"):
                            state["ps"] = mps.tile([128, 512], F32, tag="mps",
                                                   name=nm)
                        def mm(kp, wt=wt, act=act, c=c, state=state):
                            a = act[kp // 2]
                            ko = (kp % 2) * 2
                            nc.tensor.matmul(
                                state["ps"][:], wt[:, 2 * kp:2 * kp + 2, :],
                                a[:, ko:ko + 2, c * 512:(c + 1) * 512],
                                start=(kp == 0), stop=(kp == NKP - 1),
                                perf_mode=DR, skip_group_check=True,
                            )
                        def evict(dst=dst, c=c, bcol=bcol, state=state):
                            nc.vector.tensor_scalar_add(
                                dst[:, c * 512:(c + 1) * 512], state["ps"][:],
                                bqk_t[:, bcol:bcol + 1])
                        thunks.append(alloc)
                        for kp in range(NKP):
                            thunks.append(lambda kp=kp, mm=mm: mm(kp))
                        thunks.append(evict)
                return QT, KT, thunks

            oh_depth = {}

            def out_half_thunks(dc, m, n_ht):
                """Partial out-proj chunk (dc, m): heads 0..n_ht-1, evicted
                to SBUF with the bias folded in; the tail adds the rest."""
                oh_depth[(dc, m)] = n_ht
                state = {}
                th = []
                def alloc(state=state, nm=f"ohps{dc}_{m}"):
                    state["ps"] = mps.tile([128, 512], F32, tag="mps", name=nm)
                def mm(ht, dc=dc, m=m, n_ht=n_ht, state=state):
                    nc.tensor.matmul(
                        state["ps"][:], yT_t[ht][:, m * 128:(m + 1) * 128],
                        wph[dc][:, ht, :],
                        start=(ht == 0), stop=(ht == n_ht - 1),
                        skip_group_check=True,
                    )
                def evict(dc=dc, m=m, state=state):
                    nc.vector.tensor_add(
                        oh_t[dc][m][:], state["ps"][:],
                        bpb_t[:, dc * 512:(dc + 1) * 512])
                th.append(alloc)
                for ht in range(n_ht):
                    th.append(lambda ht=ht, mm=mm: mm(ht))
                th.append(evict)
                return th

            OH_SCHED = {4: [(0, 0), (0, 1), (0, 2), (0, 3), (0, 4)],
                        5: [(0, 5), (0, 6), (0, 7), (1, 0), (1, 1)],
                        6: [(1, 2), (1, 3), (1, 4), (1, 5)],
                        7: [(1, 6), (1, 7)]}

            # pair 0's projection runs un-pipelined (first PE work after warmup)
            QT, KT, th0 = qk_proj_thunks(0, wq0, wk0)
            for t in th0:
                t()

            # ---- head-pair loop, software-pipelined ----
            # v_done[vc] counts fully-emitted V chunks of each half; AV for
            # (P, j) requires chunk (P // 4, j).
            v_done = [0, 0]
            for P in range(NP):
                fillers = []
                # stage V-chunk fillers: vc0 during P0, vc1 during P1+P2
                if P == 0:
                    for zb in range(NJ):
                        fillers += v_chunk_thunks(0, zb, None)
                elif P in (1, 2):
                    if P == 1:
                        wvh1 = wf.tile([128, NK, 512], FP16, tag="wf",
                                       name="wvh1")
                        nc.sync.dma_start(wvh1[:], wv_r[:, :, 512:1024])
                    for zb in range(NJ // 2 * (P - 1), NJ // 2 * P):
                        fillers += v_chunk_thunks(1, zb, wvh1)
                elif P == 3:
                    for dc in range(2):
                        w = wf.tile([128, NK, 512], FP16, tag="wf",
                                    name=f"wph{dc}")
                        nc.sync.dma_start(w[:], wp_r[:, :, dc * 512:(dc + 1) * 512])
                        if dc == 0:
                            wph = [w]
                        else:
                            wph.append(w)
                # V-chunk bookkeeping: mark which fillers complete chunks
                v_marks = {}
                n_v = len(fillers)
                if P == 0:
                    for zb in range(NJ):
                        v_marks[(zb + 1) * 10] = (0, zb + 1)
                elif P in (1, 2):
                    base = NJ // 2 * (P - 1)
                    for i in range(NJ // 2):
                        v_marks[(i + 1) * 10] = (1, base + i + 1)

                if P + 1 < NP:
                    wqN, wkN = fetch_qk_weights(P + 1)
                    QTn, KTn, qk_fill = qk_proj_thunks(P + 1, wqN, wkN)
                    fillers += qk_fill
                for dc, m in OH_SCHED.get(P, []):
                    fillers += out_half_thunks(dc, m, NP // 2)
                fill_i = [0]

                def pop_fill(n):
                    for _ in range(n):
                        if fill_i[0] < len(fillers):
                            fillers[fill_i[0]]()
                            fill_i[0] += 1
                            if fill_i[0] in v_marks:
                                vcm, cnt = v_marks[fill_i[0]]
                                v_done[vcm] = cnt

                def pop_until_v(vc, zb):
                    # drain fillers until V chunk (vc, zb) is fully emitted
                    while v_done[vc] <= zb and fill_i[0] < n_v:
                        pop_fill(1)

                seq = [(c, j) for c in range(NC)
                       for j in range(NJ) if 128 * j <= 512 * c + 511]
                quota = -(-len(fillers) // len(seq))  # ceil: spread evenly
                last_of_c = {c: max(j for cc, j in seq if cc == c)
                             for c in range(NC)}
                yp_t = {}
                pend = []  # [(c, j, at)] awaiting AV (+ norm when last of c)
                vc_P = P // 4

                def emit_av_and_norm(c, j, at):
                    x0 = 128 * max(j - 4 * c, 0)
                    for h01 in range(2):
                        h = 2 * P + h01
                        nc.tensor.matmul(
                            yp_t[c][h01][:, x0:512],
                            V_t[vc_P][j][:, (h % 8) * 65:((h % 8) + 1) * 65],
                            at[:, h01 * 512 + x0:(h01 + 1) * 512],
                            start=(j == 0), stop=(j == last_of_c[c]),
                            skip_group_check=True,
                        )
                    if j == last_of_c[c]:
                        # normalization + eviction to packed pair layout
                        # (sumexp must round-trip through SBUF:
                        # reciprocal_approx_fast is a bitwise-seed op and
                        # PSUM's e10m23 bits are not fp32)
                        for h01 in range(2):
                            hoff = 64 * h01
                            se_t = norm.tile([1, 512], F32, tag="se")
                            nc.scalar.copy(se_t[:], yp_t[c][h01][64:65, :])
                            r_t = norm.tile([1, 512], F32, tag="rt")
                            nc.vector.reciprocal_approx_fast(r_t[:], se_t[:])
                            bc_t = norm.tile([64, 512], F32, tag="bc")
                            nc.gpsimd.partition_broadcast(bc_t[:], r_t[:])
                            nc.vector.tensor_mul(
                                yT_t[P][hoff:hoff + 64, c * 512:(c + 1) * 512],
                                yp_t[c][h01][0:64, :], bc_t[:])

                for (c, j) in seq:
                    if c not in yp_t:
                        yp_t[c] = [yps.tile([65, 512], F32, tag="yps",
                                            name=f"yp{P}_{c}_{h01}")
                                   for h01 in range(2)]
                    kband = j - 4 * c
                    x0 = 128 * max(kband, 0)
                    sp = sps.tile([128, 1024], F32, tag="sps")
                    at = apool.tile([128, 1024], FP16, tag="at")
                    for h01 in range(2):
                        hoff = 64 * h01
                        nc.tensor.matmul(
                            sp[:, h01 * 512 + x0:(h01 + 1) * 512],
                            KT[hoff:hoff + 64, j * 128:(j + 1) * 128],
                            QT[hoff:hoff + 64, c * 512 + x0:(c + 1) * 512],
                            start=True, stop=True, skip_group_check=True,
                        )
                    # one exp over both heads' regions (strided 2-bank AP)
                    sp_v = sp[:].rearrange("p (h x) -> p h x", x=512)[:, :, x0:512]
                    at_v = at[:].rearrange("p (h x) -> p h x", x=512)[:, :, x0:512]
                    nc.scalar.activation(at_v, sp_v, Exp, bias=0.0, scale=SCALE)
                    if kband >= 0:
                        at_m = at[:].rearrange(
                            "p (h x) -> p h x", x=512)[:, :, x0:x0 + 128]
                        mk_m = maskt_t[:].rearrange("p (h x) -> p h x", x=128)
                        nc.vector.tensor_mul(at_m, at_m, mk_m)
                    pop_fill(quota)
                    if len(pend) >= 2:
                        pc, pj, pat = pend.pop(0)
                        pop_until_v(vc_P, pj)
                        if pj == last_of_c[pc]:
                            # extra PE cover for the norm-chain latency the
                            # next chunk's first AV will wait on
                            pop_fill(quota)
                        emit_av_and_norm(pc, pj, pat)
                    pend.append((c, j, at))
                for pc, pj, pat in pend:
                    pop_until_v(vc_P, pj)
                    emit_av_and_norm(pc, pj, pat)
                pop_fill(len(fillers))
                if P + 1 < NP:
                    QT, KT = QTn, KTn

            # ---- output projection tail: ht 4..7 plus the prefolded half ----
            for dc in range(2):
                for m in range(NJ):
                    ps = mps.tile([128, 512], F32, tag="mps")
                    for ht in range(NP // 2, NP):
                        nc.tensor.matmul(
                            ps[:], yT_t[ht][:, m * 128:(m + 1) * 128], wph[dc][:, ht, :],
                            start=(ht == NP // 2), stop=(ht == NP - 1),
                        )
                    o_t = opool.tile([128, 512], FP16, tag="ot")
                    nc.vector.tensor_add(o_t[:], ps[:], oh_t[dc][m][:])
                    nc.sync.dma_start(
                        out_d.ap()[m * 128:(m + 1) * 128, dc * 512:(dc + 1) * 512],
                        o_t[:])

    nc.compile()
    return nc


_CACHED_NC = None


def _get_program():
    global _CACHED_NC
    if _CACHED_NC is None:
        _CACHED_NC = build_program()
    return _CACHED_NC


def _prep_shared(Wq, bq, Wk, bk, Wv, bv, Wp, bp, mask):
    assert np.array_equal(
        np.asarray(mask), np.tril(np.ones((T, T), dtype=bool))
    ), "kernel specialized for causal (tril) mask"
    wq = np.ascontiguousarray(
        np.asarray(Wq, np.float32).transpose(1, 0, 2).reshape(D, H * E).astype(E4M3))
    wk = np.ascontiguousarray(
        np.asarray(Wk, np.float32).transpose(1, 0, 2).reshape(D, H * E).astype(E4M3))
    wv = np.ascontiguousarray(
        np.asarray(Wv, np.float32).transpose(1, 0, 2).reshape(D, H * E).astype(np.float16))
    wp = np.ascontiguousarray(np.asarray(Wp, np.float32).astype(np.float16))
    bq_c = np.asarray(bq, np.float32).reshape(-1)
    bk_c = np.asarray(bk, np.float32).reshape(-1)
    bqk = np.concatenate(
        [bq_c.reshape(8, 128).T, bk_c.reshape(8, 128).T], axis=1
    ).astype(np.float32)
    tri = np.triu(np.ones((128, 128), np.float16))  # allow z <= x
    maskt = np.concatenate([tri, tri], axis=1)      # [128, 256] for both heads
    bvb = np.ascontiguousarray(np.broadcast_to(
        np.asarray(bv, np.float32).reshape(1, -1), (128, H * E)).astype(np.float16))
    bpb = np.ascontiguousarray(np.broadcast_to(
        np.asarray(bp, np.float32).reshape(1, -1), (128, H * E)).astype(np.float32))
    return {
        "wq": wq, "wk": wk, "wv": wv, "wp": wp,
        "bqk": np.ascontiguousarray(bqk),
        "bvb": bvb, "bpb": bpb,
        "maskt": np.ascontiguousarray(maskt),
    }


def kernel(x, z, Wq, bq, Wk, bk, Wv, bv, Wp, bp, mask, _trace=False, _trace_kwargs=None):
    x = np.asarray(x, np.float32)
    z = np.asarray(z, np.float32)
    shared = _prep_shared(Wq, bq, Wk, bk, Wv, bv, Wp, bp, mask)
    in_maps = []
    def tile8(a):  # [D,T] -> [128, NK*T] matching the SBUF (p, k, t) layout
        return np.ascontiguousarray(
            a.reshape(NK, 128, T).transpose(1, 0, 2).reshape(128, NK * T)
        ).astype(E4M3)

    for b in range(B):
        m = dict(shared)
        zt = np.ascontiguousarray(z[b].T)
        m["zT"] = zt.astype(np.float16)
        m["z8"] = tile8(zt)
        m["x8"] = tile8(np.ascontiguousarray(x[b].T))
        in_maps.append(m)
    nc = _get_program()
    res = run_bass_kernel_spmd(
        nc, in_maps, core_ids=list(range(B)),
        trace=_trace, **(_trace_kwargs or {}),
    )
    out = np.stack([r["out"] for r in res.results]).astype(np.float32)
    if _trace:
        kernel.last_results = res
    return out
